# revision 13
# baseline (speedup 1.0000x reference)
"""Conformer layer on 8 Trainium2 NeuronCores (bf16 datapath).

Sharding: core c handles batch b=c//2, token half sc=c%2. Each core runs
ALL 8 heads over its own 544-query window (512 own tokens + 16-token halo
each side); k/v span the full T=1024 via ONE pair AllGather of
(k_local, v_local, h_edge).  BatchNorm stats use an 8-way AllReduce.

All matmuls run in bf16 (fp32 PSUM accumulation); the residual stream
stays fp32 in SBUF.  The rel-pos bd term is computed as a banded matmul
[q, band], bounced through DRAM with a row-stride mismatch that realizes
the rel-shift, read back TRANSPOSED by the XBAR DMA engine as [k, q], and
added into the score PSUM with an identity-stationary matmul.
"""

import numpy as np

import concourse.bass as bass
import concourse.mybir as mybir
import concourse.tile as tile
from concourse import bacc
from concourse.bass import ds, ts
from concourse.bass_utils import run_bass_kernel_spmd
from contextlib import ExitStack

F32 = mybir.dt.float32
BF16 = mybir.dt.bfloat16
AF = mybir.ActivationFunctionType
ALU = mybir.AluOpType

D, DFF, H, DK, KCONV = 512, 2048, 8, 64, 31
B, T = 4, 1024
EPS = 1e-5
HT = 512
QW = 544                 # query window: 16 + 512 + 16
PW = 2304                # p_sb width (shifted/padded positions)
SW, SR = 1280, 1279      # bd dram write row stride / shear-read stride
CH = 128 * SR            # bd dram per-qchunk stride
BDH = 5 * CH             # bd dram per-head size
KVK, KVV, KVE = 4 * 128 * 512, 4 * 128 * 512, 128 * 4 * 32
CCN = KVK + KVV + KVE
NCORES = 8
DEBUG = 0

PAIRS = [[0, 1], [2, 3], [4, 5], [6, 7]]
ALLG = [[0, 1, 2, 3, 4, 5, 6, 7]]
QCW = [128, 128, 128, 128, 32]   # q-chunk widths


def _emit(nc):
    def inp(name, shape, dt=F32):
        return nc.dram_tensor(name, list(shape), dt, kind="ExternalInput")

    x_d = inp("x_loc", (4, 128, HT))
    wf1_d = inp("wf1", (4, 128, DFF), BF16); bf1_d = inp("bf1", (128, 16))
    wf2_d = inp("wf2", (16, 128, D), BF16); bf2_d = inp("bf2", (128, 4))
    wq_d = inp("wq", (4, 128, D), BF16); bq_d = inp("bq", (128, 4))
    dqv_d = inp("dqv", (128, 4))
    wk_d = inp("wk", (4, 128, D), BF16); bk_d = inp("bk", (128, 4))
    wv_d = inp("wv", (4, 128, D), BF16); bv_d = inp("bvrow", (1, D))
    wp_d = inp("wp", (4, 128, D), BF16)
    wo_d = inp("wo", (8, 64, D), BF16); bo_d = inp("bo", (128, 4))
    posT_d = inp("posT", (4, 128, PW), BF16)
    pw1_d = inp("pw1", (4, 128, 1024), BF16); bpw1_d = inp("bpw1", (128, 8))
    dwdg_d = inp("dwdiag", (4, KCONV, 128, 128), BF16)
    bng_d = inp("bng", (128, 4)); bnb_d = inp("bnb", (128, 4))
    pw2_d = inp("pw2", (4, 128, D), BF16); bpw2_d = inp("bpw2", (128, 4))
    cmask_d = inp("cmask", (1, QW))
    wg1_d = inp("wg1", (4, 128, DFF), BF16); bg1_d = inp("bg1", (128, 16))
    wg2_d = inp("wg2", (16, 128, D), BF16); bg2_d = inp("bg2", (128, 4))
    g5_d = inp("g5", (128, 4)); b5_d = inp("b5", (128, 4))
    onc_d = inp("onc", (128, 1))
    oncb_d = inp("oncb", (128, 1), BF16)
    onrb_d = inp("onrb", (1, 128), BF16)
    idn_d = inp("idn", (128, 128), BF16)

    out_d = nc.dram_tensor("out_loc", [4, 128, HT], F32, kind="ExternalOutput")
    if DEBUG:
        dbg_hw = nc.dram_tensor("dbg_hw", [4, 128, QW], F32,
                                kind="ExternalOutput")
        dbg_h2w = nc.dram_tensor("dbg_h2w", [4, 128, QW], F32,
                                 kind="ExternalOutput")
        dbg_h3 = nc.dram_tensor("dbg_h3", [4, 128, HT], F32,
                                kind="ExternalOutput")
        dbg_q = nc.dram_tensor("dbg_q", [4, 128, QW], F32,
                               kind="ExternalOutput")
        dbg_k = nc.dram_tensor("dbg_k", [4, 128, T], F32,
                               kind="ExternalOutput")
        dbg_oh = nc.dram_tensor("dbg_oh", [H, 64, QW], F32,
                                kind="ExternalOutput")
        dbg_bdt = nc.dram_tensor("dbg_bdt", [H, 128, QW], F32,
                                 kind="ExternalOutput")
        dbg_xn3 = nc.dram_tensor("dbg_xn3", [4, 128, QW], F32,
                                 kind="ExternalOutput")
        dbg_glu = nc.dram_tensor("dbg_glu", [4, 128, QW], F32,
                                 kind="ExternalOutput")
        dbg_acc = nc.dram_tensor("dbg_acc", [4, 128, HT], F32,
                                 kind="ExternalOutput")
        dbg_ysl = nc.dram_tensor("dbg_ysl", [4, 128, HT], F32,
                                 kind="ExternalOutput")
        dbg_bn = nc.dram_tensor("dbg_bn", [128, 8], F32,
                                kind="ExternalOutput")

    cc_kv_in = nc.dram_tensor("cc_kv_in", [CCN], BF16)
    cc_kv_out = nc.dram_tensor("cc_kv_out", [2, CCN], BF16)
    cc_bn_in = nc.dram_tensor("cc_bn_in", [128, 8], F32)
    cc_bn_out = nc.dram_tensor("cc_bn_out", [128, 8], F32)
    bd_d = [nc.dram_tensor(f"bd_{h}", [BDH], BF16) for h in range(H)]

    ln_uid = [0]

    with tile.TileContext(nc) as tc, ExitStack() as ctx:
        const = ctx.enter_context(tc.tile_pool(name="const", bufs=1))
        ones_c = const.tile([128, 1], F32)
        nc.gpsimd.dma_start(out=ones_c[:], in_=onc_d[:])
        ones_cb = const.tile([128, 1], BF16)
        nc.gpsimd.dma_start(out=ones_cb[:], in_=oncb_d[:])
        ones_rb = const.tile([1, 128], BF16)
        nc.gpsimd.dma_start(out=ones_rb[:], in_=onrb_d[:])
        ident = const.tile([128, 128], BF16)
        nc.gpsimd.dma_start(out=ident[:], in_=idn_d[:])
        eps1 = const.tile([1, 1], F32); nc.vector.memset(eps1[:], EPS)
        epsP = const.tile([128, 1], F32); nc.vector.memset(epsP[:], EPS)

        act = ctx.enter_context(tc.tile_pool(name="act", bufs=1))
        hw = act.tile([128, 4, QW], F32)          # h after FFN1, windowed
        p_sb = act.tile([128, 4, PW], BF16)       # positional projection
        k_full = act.tile([128, 4, T], BF16)
        v65 = act.tile([128, 8, H, 65], BF16)
        nc.vector.memset(v65[:, :, :, 64:65], 1.0)
        q_sb = act.tile([128, 4, QW], BF16)
        qv_sb = act.tile([128, 4, QW], BF16)
        xn2w = act.tile([128, 4, QW], BF16)
        h3_sb = act.tile([128, 4, HT], F32)
        h4_sb = act.tile([128, 4, HT], F32)

        # ---------- LayerNorm (channels-first; stats via ones-matmuls) ----
        def emit_ln(x4, out4, blocks, sbp):
            ln_uid[0] += 1
            with tc.tile_pool(name=f"lnps{ln_uid[0]}", bufs=1,
                              space="PSUM") as lnps:
                for b0, bw in blocks:
                    x2 = sbp.tile([128, 4, bw], BF16, tag=f"ln_sq{bw}")
                    nc.vector.tensor_mul(x2[:], x4[:, :, b0:b0 + bw],
                                         x4[:, :, b0:b0 + bw])
                    pss = lnps.tile([1, bw], F32, tag="lns")
                    psq = lnps.tile([1, bw], F32, tag="lnq")
                    for c in range(4):
                        nc.tensor.matmul(pss[:], ones_c[:],
                                         x4[:, c, b0:b0 + bw],
                                         start=(c == 0), stop=(c == 3))
                    for c in range(4):
                        nc.tensor.matmul(psq[:], ones_cb[:], x2[:, c, :],
                                         start=(c == 0), stop=(c == 3))
                    mng = sbp.tile([1, bw], F32, tag="ln_m")
                    nc.scalar.activation(mng[:], pss[:], AF.Copy,
                                         scale=-1.0 / D)
                    e2 = sbp.tile([1, bw], F32, tag="ln_e2")
                    nc.scalar.activation(e2[:], psq[:], AF.Copy, scale=1.0 / D)
                    var = sbp.tile([1, bw], F32, tag="ln_var")
                    nc.vector.tensor_mul(var[:], mng[:], mng[:])
                    nc.vector.tensor_sub(var[:], e2[:], var[:])
                    sd = sbp.tile([1, bw], F32, tag="ln_sd")
                    nc.scalar.activation(sd[:], var[:], AF.Sqrt, bias=eps1[:])
                    rec = sbp.tile([1, bw], F32, tag="ln_rs")
                    scr = sbp.tile([1, bw], F32, tag="ln_scr")
                    nc.vector.reciprocal_approx_accurate(rec[:], sd[:], scr[:])
                    nmr = sbp.tile([1, bw], F32, tag="ln_nm")
                    nc.vector.tensor_mul(nmr[:], mng[:], rec[:])
                    rb_t = sbp.tile([128, bw], F32, tag="ln_rb")
                    nc.gpsimd.partition_broadcast(rb_t[:], rec[:])
                    nb_t = sbp.tile([128, bw], F32, tag="ln_nb")
                    nc.gpsimd.partition_broadcast(nb_t[:], nmr[:])
                    for c in range(4):
                        o = out4[:, c, b0:b0 + bw]
                        nc.vector.tensor_mul(o, x4[:, c, b0:b0 + bw], rb_t[:])
                        nc.vector.tensor_add(o, o, nb_t[:])

        # ---------- FFN (silu MLP, 0.5x folded into w2) ----------
        def emit_ffn(xn, xres, out, w1dram, b1dram, w2dram, b2dram, pref):
            with tc.tile_pool(name=pref + "w", bufs=1) as wp, \
                 tc.tile_pool(name=pref + "t", bufs=3) as tp, \
                 tc.tile_pool(name=pref + "ps", bufs=2, space="PSUM") as psp, \
                 tc.tile_pool(name=pref + "ph", bufs=1, space="PSUM") as php:
                w1 = wp.tile([128, 4, DFF], BF16)
                for c in range(4):
                    nc.gpsimd.dma_start(out=w1[:, c, :], in_=w1dram[c])
                w2 = wp.tile([128, 16, D], BF16)
                for j in range(16):
                    nc.gpsimd.dma_start(out=w2[:, j, :], in_=w2dram[j])
                b1 = wp.tile([128, 16], F32)
                nc.gpsimd.dma_start(out=b1[:], in_=b1dram[:])
                b2 = wp.tile([128, 4], F32)
                nc.gpsimd.dma_start(out=b2[:], in_=b2dram[:])
                psh = php.tile([128, 4, HT], F32)
                for j in range(16):
                    psy = psp.tile([128, HT], F32, tag="psy")
                    for c in range(4):
                        nc.tensor.matmul(psy[:], w1[:, c, ts(j, 128)],
                                         xn[:, c, :],
                                         start=(c == 0), stop=(c == 3))
                    sig = tp.tile([128, HT], BF16, tag="sig")
                    nc.scalar.activation(sig[:], psy[:], AF.Sigmoid,
                                         bias=b1[:, j:j + 1])
                    y1 = tp.tile([128, HT], BF16, tag="y1")
                    nc.vector.scalar_tensor_tensor(
                        out=y1[:], in0=psy[:], scalar=b1[:, j:j + 1],
                        in1=sig[:], op0=ALU.add, op1=ALU.mult)
                    for f in range(4):
                        nc.tensor.matmul(psh[:, f, :], w2[:, j, ts(f, 128)],
                                         y1[:], start=(j == 0), stop=(j == 15))
                for c in range(4):
                    nc.vector.scalar_tensor_tensor(
                        out=out[:, c, :], in0=psh[:, c, :],
                        scalar=b2[:, c:c + 1], in1=xres[:, c, :],
                        op0=ALU.add, op1=ALU.add)

        # ================= Stage A: p-proj + LN1 + FFN1 =================
        with tc.tile_pool(name="stA", bufs=1) as stA, \
             tc.tile_pool(name="stAt", bufs=2) as stAt:
            x_sb = stA.tile([128, 4, HT], F32)
            for c in range(4):
                nc.gpsimd.dma_start(out=x_sb[:, c, :], in_=x_d[c])
            xn1 = stA.tile([128, 4, HT], BF16)
            emit_ln(x_sb[:], xn1[:], [(0, HT)], stAt)
            with tc.tile_pool(name="ppw", bufs=1) as ppw, \
                 tc.tile_pool(name="ppp", bufs=2, space="PSUM") as ppp:
                wp_sb = ppw.tile([128, 4, D], BF16)
                for c in range(4):
                    nc.gpsimd.dma_start(out=wp_sb[:, c, :], in_=wp_d[c])
                posT = ppw.tile([128, 4, PW], BF16)
                for c in range(4):
                    nc.gpsimd.dma_start(out=posT[:, c, :], in_=posT_d[c])
                PSL = [(128, 512), (640, 512), (1152, 512), (1664, 512),
                       (2176, 128)]
                for m in range(4):
                    for si, (s0, sw) in enumerate(PSL):
                        psp_t = ppp.tile([128, 512], F32, tag="psp")
                        for c in range(4):
                            nc.tensor.matmul(psp_t[0:128, 0:sw],
                                             wp_sb[:, c, ts(m, 128)],
                                             posT[:, c, s0:s0 + sw],
                                             start=(c == 0), stop=(c == 3))
                        if si % 2 == 0:
                            nc.scalar.activation(p_sb[:, m, s0:s0 + sw],
                                                 psp_t[0:128, 0:sw], AF.Copy)
                        else:
                            nc.vector.tensor_copy(p_sb[:, m, s0:s0 + sw],
                                                  psp_t[0:128, 0:sw])
            emit_ffn(xn1[:], x_sb[:], hw[:, :, 16:16 + HT],
                     wf1_d, bf1_d, wf2_d, bf2_d, "f1")

        # ============ Stage B: LN2(own) + k/v proj + AllGather ============
        with tc.tile_pool(name="stB", bufs=1) as stB, \
             tc.tile_pool(name="stBt", bufs=2) as stBt, \
             tc.tile_pool(name="qkp", bufs=2, space="PSUM") as qkps:
            emit_ln(hw[:, :, 16:16 + HT], xn2w[:, :, 16:16 + HT],
                    [(0, HT)], stBt)
            xn2o = xn2w[:, :, 16:16 + HT]
            wk_sb = stB.tile([128, 4, D], BF16)
            wv_sb = stB.tile([128, 4, D], BF16)
            wq_sb = stB.tile([128, 4, D], BF16)
            for c in range(4):
                nc.gpsimd.dma_start(out=wk_sb[:, c, :], in_=wk_d[c])
                nc.gpsimd.dma_start(out=wv_sb[:, c, :], in_=wv_d[c])
                nc.gpsimd.dma_start(out=wq_sb[:, c, :], in_=wq_d[c])
            bk_sb = stB.tile([128, 4], F32)
            nc.gpsimd.dma_start(out=bk_sb[:], in_=bk_d[:])
            bq_sb = stB.tile([128, 4], F32)
            nc.gpsimd.dma_start(out=bq_sb[:], in_=bq_d[:])
            dqv_sb = stB.tile([128, 4], F32)
            nc.gpsimd.dma_start(out=dqv_sb[:], in_=dqv_d[:])
            bv_sb = stB.tile([1, D], F32)
            nc.gpsimd.dma_start(out=bv_sb[:], in_=bv_d[:])
            bvb = stB.tile([128, D], F32)
            nc.gpsimd.partition_broadcast(bvb[:], bv_sb[:])

            k_loc = stB.tile([128, 4, HT], BF16)
            v_loc = stB.tile([128, 4, D], BF16)
            for m in range(4):
                psk = qkps.tile([128, HT], F32, tag="pqk")
                for c in range(4):
                    nc.tensor.matmul(psk[:], wk_sb[:, c, ts(m, 128)],
                                     xn2o[:, c, :],
                                     start=(c == 0), stop=(c == 3))
                nc.vector.tensor_scalar_add(k_loc[:, m, :], psk[:],
                                            bk_sb[:, m:m + 1])
            for tq in range(4):
                psv = qkps.tile([128, D], F32, tag="pqk")
                for c in range(4):
                    nc.tensor.matmul(psv[:], xn2o[:, c, ts(tq, 128)],
                                     wv_sb[:, c, :],
                                     start=(c == 0), stop=(c == 3))
                nc.vector.tensor_add(v_loc[:, tq, :], psv[:], bvb[:])
            he = stB.tile([128, 4, 32], BF16)
            nc.vector.tensor_copy(he[:, :, 0:16], hw[:, :, 16:32])
            nc.vector.tensor_copy(he[:, :, 16:32], hw[:, :, 512:528])
            for m in range(4):
                nc.gpsimd.dma_start(
                    out=bass.AP(tensor=cc_kv_in, offset=m * 65536,
                                ap=[[512, 128], [1, 512]]),
                    in_=k_loc[:, m, :])
                nc.gpsimd.dma_start(
                    out=bass.AP(tensor=cc_kv_in, offset=KVK + m * 65536,
                                ap=[[512, 128], [1, 512]]),
                    in_=v_loc[:, m, :])
            nc.gpsimd.dma_start(
                out=bass.AP(tensor=cc_kv_in, offset=KVK + KVV,
                            ap=[[128, 128], [32, 4], [1, 32]]),
                in_=he[:])
            # ---- overlap window: q-proj (own) + qv ----
            for m in range(4):
                psq = qkps.tile([128, HT], F32, tag="pqk")
                for c in range(4):
                    nc.tensor.matmul(psq[:], wq_sb[:, c, ts(m, 128)],
                                     xn2o[:, c, :],
                                     start=(c == 0), stop=(c == 3))
                nc.vector.tensor_scalar_add(q_sb[:, m, 16:16 + HT], psq[:],
                                            bq_sb[:, m:m + 1])
                nc.vector.tensor_scalar_add(qv_sb[:, m, 16:16 + HT],
                                            q_sb[:, m, 16:16 + HT],
                                            dqv_sb[:, m:m + 1])

            # ---- bd bands (chunks 1-3 pre-halo, 0/4 post-halo) ----
            with tc.tile_pool(name="bdw", bufs=2) as bdwp, \
                 tc.tile_pool(name="bdps", bufs=2, space="PSUM") as bdps:

                def emit_band(h, qc):
                    hc, hr = h // 2, 64 * (h % 2)
                    qs, cw = 128 * qc, QCW[qc]
                    base = 1040 - 128 * qc
                    bdw = bdwp.tile([128, 1152], BF16, tag="bdw")
                    for si, (off, sz) in enumerate(
                            ((0, 512), (512, 512), (1024, 128))):
                        psB = bdps.tile([128, 512], F32, tag="psB")
                        nc.tensor.matmul(psB[0:cw, 0:sz],
                                         qv_sb[hr:hr + 64, hc, qs:qs + cw],
                                         p_sb[hr:hr + 64, hc,
                                              base + off:base + off + sz],
                                         start=True, stop=True)
                        if si == 1:
                            nc.vector.tensor_copy(bdw[0:cw, off:off + sz],
                                                  psB[0:cw, 0:sz])
                        else:
                            nc.scalar.activation(bdw[0:cw, off:off + sz],
                                                 psB[0:cw, 0:sz], AF.Copy)
                    nc.sync.dma_start(
                        out=bass.AP(tensor=bd_d[h], offset=qc * CH,
                                    ap=[[SW, cw], [1, 1152]]),
                        in_=bdw[0:cw, :])

                for h in range(H):
                    for qc in (1, 2, 3):
                        emit_band(h, qc)

                nc.gpsimd.collective_compute(
                    "AllGather", ALU.bypass, ins=[cc_kv_in[:]],
                    outs=[cc_kv_out[:]], replica_groups=PAIRS)

                # ---- post-collective: gather k/v, halo h, LN2/q halo ----
                for half in range(2):
                    nc.gpsimd.dma_start(
                        out=k_full[:, :, half * 512:(half + 1) * 512],
                        in_=bass.AP(tensor=cc_kv_out, offset=half * CCN,
                                    ap=[[512, 128], [65536, 4], [1, 512]]))
                    for tq in range(4):
                        nc.gpsimd.dma_start(
                            out=v65[:, half * 4 + tq, :, 0:64],
                            in_=bass.AP(tensor=cc_kv_out,
                                        offset=half * CCN + KVK + tq * 65536,
                                        ap=[[512, 128], [1, 512]]))
                nc.gpsimd.dma_start(
                    out=hw[:, :, 0:16],
                    in_=bass.AP(tensor=cc_kv_out, offset=KVK + KVV + 16,
                                ap=[[128, 128], [32, 4], [1, 16]]))
                nc.gpsimd.dma_start(
                    out=hw[:, :, 528:544],
                    in_=bass.AP(tensor=cc_kv_out, offset=CCN + KVK + KVV,
                                ap=[[128, 128], [32, 4], [1, 16]]))
                emit_ln(hw[:], xn2w[:], [(0, 16), (528, 16)], stBt)
                for m in range(4):
                    for (qs, qn) in ((0, 16), (528, 16)):
                        psqh = qkps.tile([128, 16], F32, tag="pqk")
                        for c in range(4):
                            nc.tensor.matmul(psqh[:],
                                             wq_sb[:, c, ts(m, 128)],
                                             xn2w[:, c, qs:qs + qn],
                                             start=(c == 0), stop=(c == 3))
                        nc.vector.tensor_scalar_add(
                            q_sb[:, m, qs:qs + qn], psqh[:],
                            bq_sb[:, m:m + 1])
                        nc.vector.tensor_scalar_add(
                            qv_sb[:, m, qs:qs + qn], q_sb[:, m, qs:qs + qn],
                            dqv_sb[:, m:m + 1])
                for h in range(H):
                    emit_band(h, 0)
                    emit_band(h, 4)

        if DEBUG:
            with tc.tile_pool(name="dbg1", bufs=2) as dbp:
                for c in range(4):
                    t1 = dbp.tile([128, QW], F32, tag="d1")
                    nc.vector.tensor_copy(t1[:], hw[:, c, :])
                    nc.gpsimd.dma_start(out=dbg_hw[c], in_=t1[:])
                    t2 = dbp.tile([128, QW], F32, tag="d2")
                    nc.vector.tensor_copy(t2[:], q_sb[:, c, :])
                    nc.gpsimd.dma_start(out=dbg_q[c], in_=t2[:])
                    t3 = dbp.tile([128, T], F32, tag="d3")
                    nc.vector.tensor_copy(t3[:], k_full[:, c, :])
                    nc.gpsimd.dma_start(out=dbg_k[c], in_=t3[:])

        # ================= Stage D: attention scores per head =============
        o_h = [act.tile([64, QW], BF16, name=f"o_h{_h}", tag=f"o_h{_h}") for _h in range(H)]
        with tc.tile_pool(name="bdt", bufs=4) as bdtp, \
             tc.tile_pool(name="atp", bufs=3) as atp, \
             tc.tile_pool(name="atn", bufs=2) as atn, \
             tc.tile_pool(name="pss", bufs=2, space="PSUM") as pssp, \
             tc.tile_pool(name="psav", bufs=2, space="PSUM") as psavp:
            for h in range(H):
                hc, hr = h // 2, 64 * (h % 2)
                bdts = []
                for kc in range(8):
                    bdT = bdtp.tile([128, QW], BF16, tag="bdT")
                    nc.sync.dma_start_transpose(
                        out=bdT[:],
                        in_=bass.AP(tensor=bd_d[h], offset=127 + 128 * kc,
                                    ap=[[SR, QW], [1, 128]]))
                    bdts.append(bdT)
                    if DEBUG and kc == 0:
                        db_ = atn.tile([128, QW], F32, tag="dbgbdt")
                        nc.vector.tensor_copy(db_[:], bdT[:])
                        nc.gpsimd.dma_start(out=dbg_bdt[h], in_=db_[:])
                psA = psavp.tile([65, QW], F32, tag="psA")
                for kc in range(8):
                    psS = pssp.tile([128, QW], F32, tag="psS")
                    for (o0, on) in ((0, 512), (512, 32)):
                        nc.tensor.matmul(
                            psS[:, o0:o0 + on],
                            k_full[hr:hr + 64, hc, ts(kc, 128)],
                            q_sb[hr:hr + 64, hc, o0:o0 + on],
                            start=True, stop=False, skip_group_check=True)
                        nc.tensor.matmul(
                            psS[:, o0:o0 + on], ident[:],
                            bdts[kc][:, o0:o0 + on],
                            start=False, stop=True, skip_group_check=True)
                    probs = atp.tile([128, QW], BF16, tag="probs")
                    nc.scalar.activation(probs[:], psS[:], AF.Exp)
                    for (o0, on) in ((0, 512), (512, 32)):
                        nc.tensor.matmul(
                            psA[:, o0:o0 + on], v65[:, kc, h, :],
                            probs[:, o0:o0 + on],
                            start=(kc == 0), stop=(kc == 7),
                            skip_group_check=True)
                s65 = atn.tile([65, QW], F32, tag="s65")
                nc.scalar.activation(s65[64:65, :], psA[64:65, :], AF.Copy)
                row = atn.tile([1, QW], F32, tag="row")
                nc.gpsimd.dma_start(out=row[:], in_=s65[64:65, :])
                rec = atn.tile([1, QW], F32, tag="rec")
                scr = atn.tile([1, QW], F32, tag="scr")
                nc.vector.reciprocal_approx_accurate(rec[:], row[:], scr[:])
                rb = atn.tile([64, QW], F32, tag="rb")
                nc.gpsimd.partition_broadcast(rb[:], rec[:])
                nc.vector.tensor_mul(o_h[h][:], psA[0:64, :], rb[:])
                if DEBUG:
                    dt_ = atn.tile([64, QW], F32, tag="dbgoh")
                    nc.vector.tensor_copy(dt_[:], o_h[h][:])
                    nc.gpsimd.dma_start(out=dbg_oh[h], in_=dt_[:])

        # ---- out-projection (+ residual) -> h2w ----
        late = ctx.enter_context(tc.tile_pool(name="late", bufs=1))
        h2w = late.tile([128, 4, QW], F32)
        with tc.tile_pool(name="pso", bufs=2, space="PSUM") as psop, \
             tc.tile_pool(name="aot", bufs=1) as aot:
            wo_sb = aot.tile([64, 8, D], BF16)
            for hh in range(H):
                nc.gpsimd.dma_start(out=wo_sb[:, hh, :], in_=wo_d[hh])
            bo_sb = aot.tile([128, 4], F32)
            nc.gpsimd.dma_start(out=bo_sb[:], in_=bo_d[:])
            for f in range(4):
                pso = psop.tile([128, QW], F32, tag="pso")
                for (o0, on) in ((0, 512), (512, 32)):
                    for hh in range(H):
                        nc.tensor.matmul(
                            pso[:, o0:o0 + on], wo_sb[:, hh, ts(f, 128)],
                            o_h[hh][:, o0:o0 + on],
                            start=(hh == 0), stop=(hh == 7),
                            skip_group_check=True)
                nc.vector.scalar_tensor_tensor(
                    out=h2w[:, f, :], in0=pso[:], scalar=bo_sb[:, f:f + 1],
                    in1=hw[:, f, :], op0=ALU.add, op1=ALU.add)
                if DEBUG:
                    nc.gpsimd.dma_start(out=dbg_h2w[f], in_=h2w[:, f, :])

        # ================= Stage F: conv module =================
        with tc.tile_pool(name="stF", bufs=1) as stF, \
             tc.tile_pool(name="stFt", bufs=2) as stFt:
            xn3 = stF.tile([128, 4, QW], BF16)
            emit_ln(h2w[:], xn3[:], [(0, 272), (272, 272)], stFt)
            w1c = stF.tile([128, 4, 1024], BF16)
            for c in range(4):
                nc.gpsimd.dma_start(out=w1c[:, c, :], in_=pw1_d[c])
            bp1 = stF.tile([128, 8], F32)
            nc.gpsimd.dma_start(out=bp1[:], in_=bpw1_d[:])
            cm = stF.tile([1, QW], F32)
            nc.gpsimd.dma_start(out=cm[:], in_=cmask_d[:])
            cmb = stF.tile([128, QW], F32)
            nc.gpsimd.partition_broadcast(cmb[:], cm[:])
            glu = stF.tile([128, 4, QW], BF16)
            with tc.tile_pool(name="cvp1", bufs=1, space="PSUM") as cps:
                for m in range(4):
                    psa = cps.tile([128, 2, 512], F32, tag="psa")
                    psg = cps.tile([128, 2, 512], F32, tag="psg")
                    for half in range(2):
                        sl = slice(half * 272, (half + 1) * 272)
                        for c in range(4):
                            nc.tensor.matmul(psa[:, half, 0:272],
                                             w1c[:, c, ts(m, 128)],
                                             xn3[:, c, sl],
                                             start=(c == 0), stop=(c == 3),
                                             skip_group_check=True)
                        for c in range(4):
                            nc.tensor.matmul(psg[:, half, 0:272],
                                             w1c[:, c, 512 + m * 128:
                                                 512 + (m + 1) * 128],
                                             xn3[:, c, sl],
                                             start=(c == 0), stop=(c == 3),
                                             skip_group_check=True)
                    sg = stFt.tile([128, QW], BF16, tag="sg")
                    sgv = sg[:].rearrange("p (a w) -> p a w", a=2)
                    gluv = glu[:, m, :].rearrange("p (a w) -> p a w", a=2)
                    nc.scalar.activation(sgv, psg[:, :, 0:272], AF.Sigmoid,
                                         bias=bp1[:, 4 + m:5 + m])
                    nc.vector.scalar_tensor_tensor(
                        out=gluv, in0=psa[:, :, 0:272],
                        scalar=bp1[:, m:m + 1],
                        in1=sgv, op0=ALU.add, op1=ALU.mult)
                    nc.vector.tensor_mul(glu[:, m, :], glu[:, m, :], cmb[:])
            if DEBUG:
                with tc.tile_pool(name="dbg2", bufs=2) as dbp2:
                    for c in range(4):
                        t4 = dbp2.tile([128, QW], F32, tag="d4")
                        nc.vector.tensor_copy(t4[:], xn3[:, c, :])
                        nc.gpsimd.dma_start(out=dbg_xn3[c], in_=t4[:])
                        t5 = dbp2.tile([128, QW], F32, tag="d5")
                        nc.vector.tensor_copy(t5[:], glu[:, c, :])
                        nc.gpsimd.dma_start(out=dbg_glu[c], in_=t5[:])
            acc = stF.tile([128, 4, HT], F32)
            with tc.tile_pool(name="dgw", bufs=2) as dgw, \
                 tc.tile_pool(name="dgp", bufs=2, space="PSUM") as dgp:
                for c in range(4):
                    dg = dgw.tile([128, KCONV, 128], BF16, tag="dg")
                    nc.gpsimd.dma_start(
                        out=dg[:], in_=dwdg_d[c].rearrange("j p w -> p j w"))
                    psC = dgp.tile([128, HT], F32, tag="psC")
                    for j in range(KCONV):
                        nc.tensor.matmul(psC[:], dg[:, j, :],
                                         glu[:, c, 1 + j:1 + j + HT],
                                         start=(j == 0), stop=(j == KCONV - 1))
                    nc.scalar.activation(acc[:, c, :], psC[:], AF.Copy)
            if DEBUG:
                for c in range(4):
                    nc.gpsimd.dma_start(out=dbg_acc[c], in_=acc[:, c, :])
            bnpack = stF.tile([128, 8], F32)
            for c in range(4):
                bst_t = stFt.tile([128, 6], F32, tag="bst")
                nc.vector.bn_stats(bst_t[:], acc[:, c, :])
                mv = stFt.tile([128, 2], F32, tag="mv")
                nc.vector.bn_aggr(mv[:], bst_t[:])
                nc.vector.tensor_copy(bnpack[:, 2 * c:2 * c + 1], mv[:, 0:1])
                nc.vector.scalar_tensor_tensor(
                    out=bnpack[:, 2 * c + 1:2 * c + 2], in0=mv[:, 0:1],
                    scalar=mv[:, 0:1], in1=mv[:, 1:2],
                    op0=ALU.mult, op1=ALU.add)
            nc.gpsimd.dma_start(out=cc_bn_in[:], in_=bnpack[:])
            nc.gpsimd.collective_compute(
                "AllReduce", ALU.add, ins=[cc_bn_in[:]], outs=[cc_bn_out[:]],
                replica_groups=ALLG)
            bnar = stF.tile([128, 8], F32)
            nc.gpsimd.dma_start(out=bnar[:], in_=cc_bn_out[:])
            bng_sb = stF.tile([128, 4], F32)
            nc.gpsimd.dma_start(out=bng_sb[:], in_=bng_d[:])
            bnb_sb = stF.tile([128, 4], F32)
            nc.gpsimd.dma_start(out=bnb_sb[:], in_=bnb_d[:])
            w2c = stF.tile([128, 4, D], BF16)
            for c in range(4):
                nc.gpsimd.dma_start(out=w2c[:, c, :], in_=pw2_d[c])
            bp2 = stF.tile([128, 4], F32)
            nc.gpsimd.dma_start(out=bp2[:], in_=bpw2_d[:])
            ysl = stF.tile([128, 4, HT], BF16)
            for c in range(4):
                mg = stFt.tile([128, 1], F32, tag="mg")
                nc.scalar.activation(mg[:], bnar[:, 2 * c:2 * c + 1], AF.Copy,
                                     scale=1.0 / NCORES)
                e2 = stFt.tile([128, 1], F32, tag="e2c")
                nc.scalar.activation(e2[:], bnar[:, 2 * c + 1:2 * c + 2],
                                     AF.Copy, scale=1.0 / NCORES)
                vg = stFt.tile([128, 1], F32, tag="vg")
                nc.vector.tensor_mul(vg[:], mg[:], mg[:])
                nc.vector.tensor_sub(vg[:], e2[:], vg[:])
                sdc = stFt.tile([128, 1], F32, tag="sdc")
                nc.scalar.activation(sdc[:], vg[:], AF.Sqrt, bias=epsP[:])
                rs = stFt.tile([128, 1], F32, tag="rsc")
                nc.vector.reciprocal(rs[:], sdc[:])
                s1 = stFt.tile([128, 1], F32, tag="s1c")
                nc.vector.tensor_mul(s1[:], rs[:], bng_sb[:, c:c + 1])
                s2 = stFt.tile([128, 1], F32, tag="s2c")
                nc.vector.tensor_mul(s2[:], mg[:], s1[:])
                nc.vector.tensor_sub(s2[:], bnb_sb[:, c:c + 1], s2[:])
                sg2 = stFt.tile([128, HT], F32, tag="sg2")
                nc.scalar.activation(sg2[:], acc[:, c, :], AF.Sigmoid,
                                     scale=s1[:], bias=s2[:])
                nc.vector.tensor_scalar(ysl[:, c, :], acc[:, c, :],
                                        s1[:], s2[:], ALU.mult, ALU.add)
                nc.vector.tensor_mul(ysl[:, c, :], ysl[:, c, :], sg2[:])
            if DEBUG:
                nc.gpsimd.dma_start(out=dbg_bn[:], in_=bnar[:])
                with tc.tile_pool(name="dbg3", bufs=2) as dbp3:
                    for c in range(4):
                        t6 = dbp3.tile([128, HT], F32, tag="d6")
                        nc.vector.tensor_copy(t6[:], ysl[:, c, :])
                        nc.gpsimd.dma_start(out=dbg_ysl[c], in_=t6[:])
            with tc.tile_pool(name="cvp2", bufs=2, space="PSUM") as cps2:
                for f in range(4):
                    psw = cps2.tile([128, HT], F32, tag="psw")
                    for c in range(4):
                        nc.tensor.matmul(psw[:], w2c[:, c, ts(f, 128)],
                                         ysl[:, c, :],
                                         start=(c == 0), stop=(c == 3))
                    nc.vector.scalar_tensor_tensor(
                        out=h3_sb[:, f, :], in0=psw[:], scalar=bp2[:, f:f + 1],
                        in1=h2w[:, f, 16:16 + HT], op0=ALU.add, op1=ALU.add)

        if DEBUG:
            for c in range(4):
                nc.gpsimd.dma_start(out=dbg_h3[c], in_=h3_sb[:, c, :])

        # ================= Stage G: FFN2 =================
        with tc.tile_pool(name="stG", bufs=1) as stG, \
             tc.tile_pool(name="stGt", bufs=2) as stGt:
            xn4 = stG.tile([128, 4, HT], BF16)
            emit_ln(h3_sb[:], xn4[:], [(0, HT)], stGt)
            emit_ffn(xn4[:], h3_sb[:], h4_sb[:], wg1_d, bg1_d, wg2_d, bg2_d,
                     "f2")

        # ================= Stage H: LN5 + output =================
        with tc.tile_pool(name="stH", bufs=1) as stH, \
             tc.tile_pool(name="stHt", bufs=2) as stHt:
            g5_sb = stH.tile([128, 4], F32)
            nc.gpsimd.dma_start(out=g5_sb[:], in_=g5_d[:])
            b5_sb = stH.tile([128, 4], F32)
            nc.gpsimd.dma_start(out=b5_sb[:], in_=b5_d[:])
            xn5 = stH.tile([128, 4, HT], F32)
            emit_ln(h4_sb[:], xn5[:], [(0, HT)], stHt)
            for c in range(4):
                nc.vector.tensor_scalar(xn5[:, c, :], xn5[:, c, :],
                                        g5_sb[:, c:c + 1], b5_sb[:, c:c + 1],
                                        ALU.mult, ALU.add)
                nc.gpsimd.dma_start(out=out_d[c], in_=xn5[:, c, :])
    return nc


_CACHE = {}


def build_nc():
    if "nc" not in _CACHE:
        nc = bacc.Bacc("TRN2", target_bir_lowering=False, debug=False,
                       num_devices=NCORES)
        _emit(nc)
        nc.compile()
        _CACHE["nc"] = nc
    return _CACHE["nc"]


def _bf(a):
    import ml_dtypes
    return np.ascontiguousarray(np.asarray(a, dtype=np.float32)
                                .astype(ml_dtypes.bfloat16))


def _chunk_cf(a2d):
    """[Din, W] -> [Din//128, 128, W] chunk-major channels-first."""
    d, w = a2d.shape
    return np.ascontiguousarray(
        np.asarray(a2d, dtype=np.float32).reshape(d // 128, 128, w))


def _pcol(vec):
    """[Dout] per-channel vector -> [128, Dout//128]."""
    n = vec.shape[0]
    return np.ascontiguousarray(np.asarray(vec, np.float32)
                                .reshape(n // 128, 128).T)


def make_in_maps(inputs):
    inputs = {k: np.asarray(v, dtype=np.float32) for k, v in inputs.items()}
    x = inputs["x"]; pos_emb = inputs["pos_emb"]
    ln1_g, ln1_b = inputs["ln1_g"], inputs["ln1_b"]
    ln2_g, ln2_b = inputs["ln2_g"], inputs["ln2_b"]
    ln3_g, ln3_b = inputs["ln3_g"], inputs["ln3_b"]
    ln4_g, ln4_b = inputs["ln4_g"], inputs["ln4_b"]

    w1f = ln1_g[:, None] * inputs["ff1_w1"]
    b1f = inputs["ff1_b1"] + ln1_b @ inputs["ff1_w1"]
    w2f = 0.5 * inputs["ff1_w2"]; b2f = 0.5 * inputs["ff1_b2"]
    wg1f = ln4_g[:, None] * inputs["ff2_w1"]
    bg1f = inputs["ff2_b1"] + ln4_b @ inputs["ff2_w1"]
    wg2f = 0.5 * inputs["ff2_w2"]; bg2f = 0.5 * inputs["ff2_b2"]

    s = DK ** -0.5
    pos_u_f = inputs["pos_u"].reshape(D); pos_v_f = inputs["pos_v"].reshape(D)
    wqf = s * (ln2_g[:, None] * inputs["wq"])
    bqf = s * (inputs["bq"] + ln2_b @ inputs["wq"] + pos_u_f)
    dqvf = s * (pos_v_f - pos_u_f)
    wkf = ln2_g[:, None] * inputs["wk"]
    bkf = inputs["bk"] + ln2_b @ inputs["wk"]
    wvf = ln2_g[:, None] * inputs["wv"]
    bvf = inputs["bv"] + ln2_b @ inputs["wv"]

    pw1f = (inputs["pw1_w"] * ln3_g[None, :]).T            # [512, 1024]
    bpw1f = inputs["pw1_b"] + inputs["pw1_w"] @ ln3_b      # [1024]
    dwwf = inputs["dw_w"][:, 0, :]                         # [512, 31]
    dwdiag = np.zeros((4, KCONV, 128, 128), dtype=np.float32)
    ar = np.arange(128)
    for c4 in range(4):
        for j4 in range(KCONV):
            dwdiag[c4, j4, ar, ar] = dwwf[c4 * 128:(c4 + 1) * 128, j4]
    pw2f = inputs["pw2_w"].T                               # [512, 512]

    # posT variants: col j holds pos_emb position (j - off), off=128+512*sc
    pT = pos_emb[0].T                                      # [512, 2047]
    posT_sc = []
    for sc in range(2):
        off = 128 + 512 * sc
        buf = np.zeros((D, PW), dtype=np.float32)
        lo, hi = off, min(PW, off + 2 * T - 1)
        buf[:, lo:hi] = pT[:, 0:hi - lo]
        posT_sc.append(_bf(_chunk_cf(buf)))

    base = {
        "wf1": _bf(_chunk_cf(w1f)), "bf1": _pcol(b1f),
        "wf2": _bf(_chunk_cf(w2f)), "bf2": _pcol(b2f),
        "wg1": _bf(_chunk_cf(wg1f)), "bg1": _pcol(bg1f),
        "wg2": _bf(_chunk_cf(wg2f)), "bg2": _pcol(bg2f),
        "wq": _bf(_chunk_cf(wqf)), "bq": _pcol(bqf), "dqv": _pcol(dqvf),
        "wk": _bf(_chunk_cf(wkf)), "bk": _pcol(bkf),
        "wv": _bf(_chunk_cf(wvf)),
        "bvrow": np.ascontiguousarray(bvf.reshape(1, D), dtype=np.float32),
        "wp": _bf(_chunk_cf(inputs["wp"])),
        "wo": _bf(np.ascontiguousarray(
            inputs["wo"].reshape(8, 64, D), dtype=np.float32)),
        "bo": _pcol(inputs["bo"]),
        "pw1": _bf(_chunk_cf(pw1f)), "bpw1": _pcol(bpw1f),
        "dwdiag": _bf(dwdiag),
        "bng": _pcol(inputs["bn_g"]), "bnb": _pcol(inputs["bn_b"]),
        "pw2": _bf(_chunk_cf(pw2f)), "bpw2": _pcol(inputs["pw2_b"]),
        "g5": _pcol(inputs["ln5_g"]), "b5": _pcol(inputs["ln5_b"]),
        "onc": np.ones((128, 1), dtype=np.float32),
        "oncb": _bf(np.ones((128, 1))),
        "onrb": _bf(np.ones((1, 128))),
        "idn": _bf(np.eye(128)),
    }

    in_maps = []
    for c in range(NCORES):
        b, sc = c // 2, c % 2
        m = dict(base)
        xb = x[b, sc * HT:(sc + 1) * HT, :].T               # [512, 512]
        m["x_loc"] = _chunk_cf(xb)
        m["posT"] = posT_sc[sc]
        cmask = np.ones((1, QW), dtype=np.float32)
        if sc == 0:
            cmask[0, :16] = 0.0
        else:
            cmask[0, QW - 16:] = 0.0
        m["cmask"] = cmask
        in_maps.append(m)
    return in_maps


def assemble_out(results):
    out = np.empty((B, T, D), dtype=np.float32)
    for c in range(NCORES):
        b, sc = c // 2, c % 2
        ol = np.asarray(results[c]["out_loc"])              # [4, 128, 512]
        out[b, sc * HT:(sc + 1) * HT, :] = ol.reshape(D, HT).T
    return out


def kernel(**inputs):
    in_maps = make_in_maps(inputs)
    nc = build_nc()
    res = run_bass_kernel_spmd(nc, in_maps, list(range(NCORES)))
    return assemble_out(res.results)
